# revision 9
# baseline (speedup 1.0000x reference)
"""Decoder block Bass/Tile kernel for TRN2, SPMD over 8 cores.

Sharding: core c = (batch b = c//4, j = c%4). Each core:
  - owns 512 query rows of its batch: chunk A = [256j, 256j+256),
    chunk B = [256(7-j), 256(7-j)+256)  (causal zigzag load balance)
  - computes LN1 + K,V for exactly its own 512 rows, then AllGathers
    K,V across the 4 cores of its batch (replica groups [0-3], [4-7])
  - attention klen padded to a uniform size (1024 for A, 2048 for B) with
    host-provided -60000 masks so the program is identical on all cores
  - proj + residual + LN2 + MLP + residual for its 512 rows
Host gathers the 8 [512, 1024] shards into the full output.

Layouts: "fm" = [feature(partition), token(free)], "rm" = [token, feature].
LN in rm (bn_stats), matmul inputs fm via fp16 DMA-transpose. Matmuls fp16
with fp32 PSUM accumulation. Residual stream fp32.
"""

from contextlib import ExitStack
from dataclasses import dataclass

import numpy as np

import concourse.bass as bass
import concourse.tile as tile
from concourse import mybir
from concourse._compat import with_exitstack

F32 = mybir.dt.float32
F16 = mybir.dt.float16
MASK_NEG = -60000.0


@dataclass
class Cfg:
    D: int = 1024
    DFF: int = 4096
    H: int = 16  # heads
    DH: int = 64  # head dim
    T_kv: int = 2048
    T_q: int = 512  # 2 chunks of CH
    CH: int = 256
    klenA_pad: int = 1024
    klenB_pad: int = 2048
    mmdt: str = "float16"

    @property
    def HP(self):  # head pairs
        return self.H // 2

    @property
    def VA(self):  # augmented V width (dv + ones column per head)
        return self.H * (self.DH + 1)

    @property
    def NKTA(self):
        return self.klenA_pad // 128

    @property
    def NKTB(self):
        return self.klenB_pad // 128

    @property
    def NKT(self):
        return self.NKTA + self.NKTB


def _bcast_ap(ap, p=128):
    """[N] dram AP -> [p, N] with partition stride 0."""
    return bass.AP(tensor=ap.tensor, offset=ap.offset, ap=[[0, p]] + list(ap.ap))


@with_exitstack
def decoder_kernel(ctx: ExitStack, tc: tile.TileContext, cfg: Cfg, io: dict):
    nc = tc.nc
    MD = getattr(mybir.dt, cfg.mmdt)
    D, DFF, H, DH = cfg.D, cfg.DFF, cfg.H, cfg.DH
    HP, VA, CH = cfg.HP, cfg.VA, cfg.CH
    T_kv, T_q = cfg.T_kv, cfg.T_q
    ND = D // 128  # feature tiles
    NFF = DFF // 128
    NTKV = T_kv // 128
    NTQ = T_q // 128
    W2 = 2 * CH  # paired-head free width (512)
    RANKS = 4  # cores per batch group

    # V chunk width for psum (<=512); VA = H*65
    n_vch = (VA + 511) // 512
    while VA % n_vch != 0:
        n_vch += 1
    VCH = VA // n_vch
    assert VCH <= 512

    const = ctx.enter_context(tc.tile_pool(name="const", bufs=1))
    eps_t = const.tile([128, 1], F32)
    nc.vector.memset(eps_t, 1e-5)
    bq_sb = const.tile([128, ND], F32)
    nc.gpsimd.dma_start(out=bq_sb, in_=io["bq"].rearrange("(t p) -> p t", p=128))
    bk_sb = const.tile([128, ND], F32)
    nc.gpsimd.dma_start(out=bk_sb, in_=io["bk"].rearrange("(t p) -> p t", p=128))
    bfc1_sb = const.tile([128, NFF], F32)
    nc.gpsimd.dma_start(out=bfc1_sb, in_=io["bfc1"].rearrange("(t p) -> p t", p=128))
    vb_sb = const.tile([128, VA], F32)
    nc.gpsimd.dma_start(out=vb_sb, in_=_bcast_ap(io["vb"]))

    # ---------------- persistent activations ----------------
    acts = ctx.enter_context(tc.tile_pool(name="acts", bufs=1))
    K_sb = [acts.tile([128, T_kv], MD, tag=f"K{d}", name=f"K{d}") for d in range(ND)]
    Q_sb = [acts.tile([128, 2 * T_q], MD, tag=f"Q{d}", name=f"Q{d}") for d in range(ND)]
    for d in range(ND):
        nc.vector.memset(Q_sb[d], 0.0)
    V_all = acts.tile([128, NTKV, VA], MD, tag="Vall", name="Vall")
    O_sb = [acts.tile([128, T_q], MD, tag=f"O{h}", name=f"O{h}") for h in range(HP)]

    # AllGather bounce buffers (DRAM): K [1024,512] + V [512,1040] packed
    # into one flat buffer so a SINGLE collective ships both (two
    # back-to-back collectives serialize: the gpsimd wait on the first
    # blocks the second's trigger).
    KELEM = D * 512  # 524288
    VELEM = T_q * VA  # 532480
    KVELEM = KELEM + VELEM  # 1056768
    assert KVELEM % 512 == 0
    kv_in = nc.dram_tensor("kv_in", [KVELEM // 512, 512], MD).ap()
    kv_out = nc.dram_tensor("kv_out", [RANKS * KVELEM // 512, 512], MD).ap()
    GROUPS = [[0, 1, 2, 3], [4, 5, 6, 7]]

    # ---------------- LN + transpose helper ----------------
    def ln_transpose(src_dram, src_sb, n_tiles, fm_tiles, pool, stats, tagp):
        for rt in range(n_tiles):
            if src_dram is not None:
                x_t = pool.tile([128, D], F32, tag=f"{tagp}_in")
                nc.gpsimd.dma_start(
                    out=x_t, in_=src_dram[rt * 128 : (rt + 1) * 128, :]
                )
            else:
                x_t = src_sb[rt]
            nsub = D // 512
            st = stats.tile([128, nsub, 6], F32, tag="ln_st")
            for s in range(nsub):
                nc.vector.bn_stats(
                    out=st[:, s, :], in_=x_t[:, s * 512 : (s + 1) * 512]
                )
            mv = stats.tile([128, 2], F32, tag="ln_mv")
            nc.vector.bn_aggr(out=mv, in_=st)
            sd = stats.tile([128, 1], F32, tag="ln_sd")
            nc.scalar.activation(
                out=sd, in_=mv[:, 1:2],
                func=mybir.ActivationFunctionType.Sqrt, bias=eps_t,
            )
            rec = stats.tile([128, 1], F32, tag="ln_rec")
            nc.vector.reciprocal(out=rec, in_=sd)
            xh = pool.tile([128, D], MD, tag=f"{tagp}_xh")
            nc.vector.tensor_scalar(
                out=xh, in0=x_t, scalar1=mv[:, 0:1], scalar2=rec,
                op0=mybir.AluOpType.subtract, op1=mybir.AluOpType.mult,
            )
            for d in range(ND):
                dst = fm_tiles[d][rt // 4][:, (rt % 4) * 128 : (rt % 4 + 1) * 128]
                eng = nc.sync if (rt * ND + d) % 2 == 0 else nc.scalar
                eng.dma_start_transpose(
                    out=dst, in_=xh[:, d * 128 : (d + 1) * 128]
                )

    with tc.tile_pool(name="fm", bufs=1) as fmp:
        xq_fm = [
            [fmp.tile([128, 512], MD, tag=f"xqfm{d}_0", name=f"xqfm{d}_0")]
            for d in range(ND)
        ]
        with tc.tile_pool(name="wqk", bufs=1) as wqk, tc.tile_pool(
            name="wv", bufs=1
        ) as wvp, tc.tile_pool(name="kvloc", bufs=1) as kvp, tc.tile_pool(
            name="psqkv", bufs=4, space="PSUM"
        ) as psq:
            # LN1 first: its x_q loads must be at the HEAD of the gpsimd
            # DMA queue (weights queue behind them) so K/V/AG start early
            with tc.tile_pool(name="ln1", bufs=3) as lnp, tc.tile_pool(
                name="ln1st", bufs=4
            ) as lnst:
                ln_transpose(io["x_q"], None, NTQ, xq_fm, lnp, lnst, "q")

            wv_sb = [wvp.tile([128, VA], MD, tag=f"wv{kt}", name=f"wv{kt}") for kt in range(ND)]
            for kt in range(ND):
                nc.gpsimd.dma_start(
                    out=wv_sb[kt], in_=io["wv"][kt * 128 : (kt + 1) * 128, :]
                )
            wk_sb = [wqk.tile([128, ND, 128], MD, tag=f"wk{do}", name=f"wk{do}") for do in range(ND)]
            wq_sb = [wqk.tile([128, ND, 128], MD, tag=f"wq{do}", name=f"wq{do}") for do in range(ND)]
            for do in range(ND):
                nc.gpsimd.dma_start(
                    out=wk_sb[do],
                    in_=io["wk"][:, do * 128 : (do + 1) * 128].rearrange(
                        "(kt p) c -> p kt c", p=128
                    ),
                )
            for do in range(ND):
                nc.gpsimd.dma_start(
                    out=wq_sb[do],
                    in_=io["wq"][:, do * 128 : (do + 1) * 128].rearrange(
                        "(kt p) c -> p kt c", p=128
                    ),
                )
            kloc = kvp.tile([128, ND, 512], MD, tag="kloc", name="kloc")
            vloc = kvp.tile([128, NTQ, VA], MD, tag="vloc", name="vloc")

            # V local (own 512 tokens, rm layout)
            for tt in range(NTQ):
                for ch in range(n_vch):
                    ps = psq.tile([128, VCH], F32, tag="psv")
                    for kt in range(ND):
                        nc.tensor.matmul(
                            ps,
                            xq_fm[kt][0][:, tt * 128 : (tt + 1) * 128],
                            wv_sb[kt][:, ch * VCH : (ch + 1) * VCH],
                            start=(kt == 0),
                            stop=(kt == ND - 1),
                        )
                    nc.vector.tensor_add(
                        out=vloc[:, tt, ch * VCH : (ch + 1) * VCH],
                        in0=ps,
                        in1=vb_sb[:, ch * VCH : (ch + 1) * VCH],
                    )
            # K local (own 512 tokens, fm layout)
            for do in range(ND):
                ps = psq.tile([128, 512], F32, tag="psqk")
                for kt in range(ND):
                    nc.tensor.matmul(
                        ps, wk_sb[do][:, kt, :], xq_fm[kt][0],
                        start=(kt == 0), stop=(kt == ND - 1),
                    )
                nc.scalar.activation(
                    out=kloc[:, do, :], in_=ps,
                    func=mybir.ActivationFunctionType.Identity,
                    bias=bk_sb[:, do : do + 1],
                )

            # ship local K,V to DRAM and AllGather within the batch group
            nc.gpsimd.dma_start(
                out=bass.AP(
                    tensor=kv_in.tensor, offset=kv_in.offset,
                    ap=[[512, 128], [128 * 512, ND], [1, 512]],
                ),
                in_=kloc,
            )
            nc.gpsimd.dma_start(
                out=bass.AP(
                    tensor=kv_in.tensor, offset=kv_in.offset + KELEM,
                    ap=[[VA, 128], [128 * VA, NTQ], [1, VA]],
                ),
                in_=vloc,
            )
            nc.gpsimd.collective_compute(
                "AllGather", mybir.AluOpType.bypass, replica_groups=GROUPS,
                ins=[kv_in], outs=[kv_out],
            )

            # Q (overlaps the AllGather; no gpsimd involvement)
            for do in range(ND):
                ps = psq.tile([128, 512], F32, tag="psqk")
                for kt in range(ND):
                    nc.tensor.matmul(
                        ps, wq_sb[do][:, kt, :], xq_fm[kt][0],
                        start=(kt == 0), stop=(kt == ND - 1),
                    )
                # Q: scatter into per-(chunk, head) blocks with the
                # complementary head's partitions left zero
                for ci in range(2):
                    for h in range(2):
                        blk = (2 * ci + h) * CH
                        nc.scalar.activation(
                            out=Q_sb[do][
                                h * 64 : (h + 1) * 64,
                                blk : blk + CH,
                            ],
                            in_=ps[
                                h * 64 : (h + 1) * 64,
                                ci * CH : (ci + 1) * CH,
                            ],
                            func=mybir.ActivationFunctionType.Identity,
                            bias=bq_sb[h * 64 : (h + 1) * 64, do : do + 1],
                        )

            # readback: gathered K into K_sb (global token order), V into V_all
            # rank r local tokens = [A: 256r..256r+256) | B: 256(7-r)..+256)
            # spread across 4 engine queues so issue doesn't serialize
            rr_engs = [nc.gpsimd, nc.sync, nc.scalar]
            rr = 0

            def rb_dma(out, in_):
                nonlocal rr
                rr_engs[rr % 3].dma_start(out=out, in_=in_)
                rr += 1

            for do in range(ND):
                # A chunks of all 4 ranks, one 3D DMA
                src = bass.AP(
                    tensor=kv_out.tensor,
                    offset=kv_out.offset + do * 128 * 512,
                    ap=[[512, 128], [KVELEM, RANKS], [1, 256]],
                )
                rb_dma(
                    K_sb[do][:, 0:1024].rearrange("p (r c) -> p r c", r=4),
                    src,
                )
                for r in range(RANKS):
                    src = bass.AP(
                        tensor=kv_out.tensor,
                        offset=kv_out.offset
                        + r * KVELEM
                        + do * 128 * 512
                        + 256,
                        ap=[[512, 128], [1, 256]],
                    )
                    rb_dma(K_sb[do][:, 256 * (7 - r) : 256 * (8 - r)], src)
            for r in range(RANKS):
                for half, t0 in ((0, 2 * r), (1, 14 - 2 * r)):
                    src = bass.AP(
                        tensor=kv_out.tensor,
                        offset=kv_out.offset
                        + r * KVELEM
                        + KELEM
                        + half * 256 * VA,
                        ap=[[VA, 128], [128 * VA, 2], [1, VA]],
                    )
                    rb_dma(V_all[:, t0 : t0 + 2, :], src)

    # ---------------- attention + proj ----------------
    mid = ctx.enter_context(tc.tile_pool(name="mid", bufs=1))
    x2_sb = [mid.tile([128, D], F32, tag=f"x2_{t}", name=f"x2_{t}") for t in range(NTQ)]
    xq2_fm = [
        [mid.tile([128, 512], MD, tag=f"xq2fm{d}_{c}", name=f"xq2fm{d}_{c}")
         for c in range(T_q // 512)]
        for d in range(ND)
    ]
    rscr = nc.dram_tensor("rscratch", [2 * HP * 2, CH], F32).ap()
    chunks = [(0, cfg.NKTA, 0), (1, cfg.NKTB, cfg.NKTA)]  # (ci, nkt, mask_off)
    with tc.tile_pool(name="attn_w", bufs=1) as awp:
        # prefetch wproj while attention runs (sync queue: gpsimd is
        # blocked on the collective wait around this point)
        wproj_sb = [awp.tile([128, D], MD, tag=f"wp{d}", name=f"wp{d}") for d in range(ND)]
        for d in range(ND):
            nc.sync.dma_start(
                out=wproj_sb[d], in_=io["wproj"][d * 128 : (d + 1) * 128, :]
            )
        with tc.tile_pool(name="attn_m", bufs=1) as mp, tc.tile_pool(
            name="attn_p", bufs=4
        ) as pp, tc.tile_pool(name="attn_ps", bufs=4, space="PSUM"
        ) as aps, tc.tile_pool(name="attn_po", bufs=4, space="PSUM"
        ) as ops:
            for ci, nkt, moff in chunks:
                cc = slice(ci * CH, (ci + 1) * CH)
                masks = []
                for k in range(nkt):
                    m = mp.tile([128, W2], MD, tag=f"mask{ci}_{k}")
                    nc.sync.dma_start(out=m, in_=io["masks"][moff + k, :, :])
                    masks.append(m)
                for hp in range(HP):
                    po = [ops.tile([128, CH], F32, tag="po", name="po") for _ in range(2)]
                    for kti in range(nkt):
                        ps = aps.tile([128, W2], F32, tag="ps_s")
                        kcol = slice(kti * 128, (kti + 1) * 128)
                        nc.tensor.matmul(
                            ps,
                            K_sb[hp][:, kcol],
                            Q_sb[hp][:, 2 * ci * CH : 2 * ci * CH + W2],
                            start=True, stop=True,
                        )
                        if not (ci == 1 and (kti + 1) * 128 <= cfg.klenB_pad // 2):
                            nc.vector.tensor_add(
                                out=ps, in0=ps, in1=masks[kti]
                            )
                        pt = pp.tile([128, W2], MD, tag="pt")
                        nc.scalar.activation(
                            out=pt, in_=ps,
                            func=mybir.ActivationFunctionType.Exp,
                        )
                        for h in range(2):
                            hg = 2 * hp + h
                            nc.tensor.matmul(
                                po[h][0:65, :],
                                V_all[:, kti, hg * 65 : hg * 65 + 65],
                                pt[:, h * CH : (h + 1) * CH],
                                start=(kti == 0),
                                stop=(kti == nkt - 1),
                            )
                    # normalize + evict (inline: the DRAM broadcast
                    # round-trip hides behind the next hp's attention)
                    for h in range(2):
                        slot = (ci * HP + hp) * 2 + h
                        r = pp.tile([1, CH], F32, tag="recip")
                        nc.vector.reciprocal(out=r, in_=po[h][64:65, :])
                        nc.sync.dma_start(
                            out=rscr[slot : slot + 1, :], in_=r
                        )
                        # evict numerator scaled by 1/4096 (fits fp16)
                        nc.scalar.activation(
                            out=O_sb[hp][h * 64 : (h + 1) * 64, cc],
                            in_=po[h][0:64, :],
                            func=mybir.ActivationFunctionType.Copy,
                            scale=1.0 / 4096.0,
                        )
                    bc_sb = pp.tile([128, CH], F32, tag="bcsb")
                    for h in range(2):
                        slot = (ci * HP + hp) * 2 + h
                        nc.sync.dma_start(
                            out=bc_sb[h * 64 : (h + 1) * 64, :],
                            in_=bass.AP(
                                tensor=rscr.tensor,
                                offset=rscr.offset + slot * CH,
                                ap=[[0, 64], [1, CH]],
                            ),
                        )
                    nc.vector.tensor_mul(
                        out=O_sb[hp][:, cc], in0=O_sb[hp][:, cc], in1=bc_sb
                    )

        # ---------------- proj + residual ----------------
        with tc.tile_pool(name="proj", bufs=3) as prp, tc.tile_pool(
            name="projps", bufs=4, space="PSUM"
        ) as prps:
            for qt in range(NTQ):
                x_t = prp.tile([128, D], F32, tag="xq_res")
                nc.gpsimd.dma_start(
                    out=x_t, in_=io["x_q"][qt * 128 : (qt + 1) * 128, :]
                )
                for ch2 in range(D // 512):
                    ps = prps.tile([128, 512], F32, tag="pspr")
                    for hp in range(ND):
                        nc.tensor.matmul(
                            ps,
                            O_sb[hp][:, qt * 128 : (qt + 1) * 128],
                            wproj_sb[hp][:, ch2 * 512 : (ch2 + 1) * 512],
                            start=(hp == 0),
                            stop=(hp == ND - 1),
                        )
                    nc.vector.tensor_add(
                        out=x2_sb[qt][:, ch2 * 512 : (ch2 + 1) * 512],
                        in0=ps,
                        in1=x_t[:, ch2 * 512 : (ch2 + 1) * 512],
                    )

    # ---------------- LN2 + transpose ----------------
    with tc.tile_pool(name="ln2", bufs=3) as ln2p, tc.tile_pool(
        name="ln2st", bufs=4
    ) as ln2st:
        ln_transpose(None, x2_sb, NTQ, xq2_fm, ln2p, ln2st, "l2")

    # ---------------- fc1 + gelu + fc2 (pipelined) ----------------
    ghp = ctx.enter_context(tc.tile_pool(name="gh", bufs=1))
    gh_sb = [ghp.tile([128, T_q], MD, tag=f"gh{f}", name=f"gh{f}") for f in range(NFF)]
    with tc.tile_pool(name="fc1w", bufs=3) as f1w, tc.tile_pool(
        name="fc2w", bufs=3
    ) as f2w, tc.tile_pool(name="fc2out", bufs=3) as f2o, tc.tile_pool(
        name="fcps", bufs=3, space="PSUM"
    ) as fps, tc.tile_pool(name="fc2acc", bufs=1, space="PSUM") as f2ps:
        for sweep in range(2):
            accs = {}
            for qt in range(NTQ):
                accs[qt] = f2ps.tile(
                    [128, 512], F32, tag=f"acc{qt}", name=f"acc{qt}"
                )
            for ff in range(NFF):
                if sweep == 0:
                    wb = f1w.tile([128, ND, 128], MD, tag="wfc1")
                    nc.gpsimd.dma_start(
                        out=wb,
                        in_=io["wfc1"][:, ff * 128 : (ff + 1) * 128].rearrange(
                            "(kt p) c -> p kt c", p=128
                        ),
                    )
                    ps = fps.tile([128, T_q], F32, tag="psf1")
                    for kt in range(ND):
                        nc.tensor.matmul(
                            ps, wb[:, kt, :], xq2_fm[kt][0],
                            start=(kt == 0), stop=(kt == ND - 1),
                        )
                    nc.scalar.activation(
                        out=gh_sb[ff], in_=ps,
                        func=mybir.ActivationFunctionType.Gelu,
                        bias=bfc1_sb[:, ff : ff + 1],
                    )
                wb2 = f2w.tile([128, 512], MD, tag="wfc2")
                nc.gpsimd.dma_start(
                    out=wb2,
                    in_=io["wfc2"][
                        ff * 128 : (ff + 1) * 128, sweep * 512 : (sweep + 1) * 512
                    ],
                )
                for qt in range(NTQ):
                    nc.tensor.matmul(
                        accs[qt],
                        gh_sb[ff][:, qt * 128 : (qt + 1) * 128],
                        wb2,
                        start=(ff == 0),
                        stop=(ff == NFF - 1),
                    )
            for qt in range(NTQ):
                o = f2o.tile([128, 512], F32, tag="osb")
                nc.vector.tensor_add(
                    out=o,
                    in0=accs[qt],
                    in1=x2_sb[qt][:, sweep * 512 : (sweep + 1) * 512],
                )
                nc.sync.dma_start(
                    out=io["out"][
                        qt * 128 : (qt + 1) * 128,
                        sweep * 512 : (sweep + 1) * 512,
                    ],
                    in_=o,
                )


def split_drain_waits(nc):
    """walrus CoreV3 rejects >1 sync wait on several instruction types;
    split extras into single-wait NOPs preceding the instruction on the
    same (in-order) engine."""
    idx = 0

    def fix_block(b):
        nonlocal idx
        new = []
        changed = False
        for inst in b.instructions:
            si = inst.sync_info
            if si is not None and si.on_wait and len(si.on_wait) > 1:
                waits = list(si.on_wait)
                for w in waits[:-1]:
                    idx += 1
                    nop = mybir.InstNoOp(
                        name=f"I-dsplit-{idx}",
                        sync_info=mybir.SyncInfo(on_wait=[w], on_update=[]),
                    )
                    nop.engine = inst.engine
                    new.append(nop)
                inst.sync_info = mybir.SyncInfo(
                    on_wait=[waits[-1]], on_update=list(si.on_update or [])
                )
                changed = True
            new.append(inst)
        if changed:
            b.instructions = new

    for f in nc.m.functions:
        for b in f.blocks:
            fix_block(b)


def declare_io(nc, cfg: Cfg):
    c = cfg
    WD = getattr(mybir.dt, c.mmdt)
    spec = {
        "x_q": ([c.T_q, c.D], F32, False),
        "wq": ([c.D, c.D], WD, False),
        "wk": ([c.D, c.D], WD, False),
        "wv": ([c.D, c.VA], WD, False),
        "bq": ([c.D], F32, False),
        "bk": ([c.D], F32, False),
        "vb": ([c.VA], F32, False),
        "wproj": ([c.D, c.D], WD, False),
        "wfc1": ([c.D, c.DFF], WD, False),
        "bfc1": ([c.DFF], F32, False),
        "wfc2": ([c.DFF, c.D], WD, False),
        "masks": ([c.NKT, 128, 2 * c.CH], WD, False),
        "out": ([c.T_q, c.D], F32, True),
    }
    io = {}
    for name, (shape, dt, is_out) in spec.items():
        io[name] = nc.declare_dram_parameter(name, shape, dt, isOutput=is_out).ap()
    return io


def build(cfg: Cfg, split: bool = True):
    nc = bass.Bass(num_devices=8)
    io = declare_io(nc, cfg)
    with tile.TileContext(nc) as tc:
        decoder_kernel(tc, cfg, io)
    if split:
        split_drain_waits(nc)
    return nc


# ======================= host-side prep =======================


def make_masks(cfg: Cfg, qgA, qgB):
    """[NKT, 128, 2*CH] fp16-ish: 0 where key k <= query q (valid), else
    -60000. Duplicated for the 2 heads along the free dim."""
    m = np.full((cfg.NKT, 128, 2 * cfg.CH), MASK_NEG, np.float32)
    for ci, (qg, nkt, off) in enumerate(
        [(qgA, cfg.NKTA, 0), (qgB, cfg.NKTB, cfg.NKTA)]
    ):
        q = qg + np.arange(cfg.CH)[None, :]
        for k in range(nkt):
            kg = k * 128 + np.arange(128)[:, None]
            valid = (kg <= q).astype(np.float32)
            blk = (1.0 - valid) * MASK_NEG
            m[off + k, :, 0 : cfg.CH] = blk
            m[off + k, :, cfg.CH : 2 * cfg.CH] = blk
    return m.astype(np.float16)


def host_prep(cfg: Cfg, x, ln1_g, ln1_b, w_qkv, w_proj, ln2_g, ln2_b, w_fc1, w_fc2):
    """Returns (in_maps list of 8 dicts, assemble(results)->full out)."""
    D, H, DH = cfg.D, cfg.H, cfg.DH
    x = np.asarray(x, np.float32)
    B = x.shape[0]
    w_qkv = np.asarray(w_qkv, np.float32)
    bqkv = np.asarray(ln1_b, np.float32) @ w_qkv  # [3D]
    w_qkv = w_qkv * np.asarray(ln1_g, np.float32)[:, None]
    bq = bqkv[0:D] / np.sqrt(DH).astype(np.float32)
    bk = bqkv[D : 2 * D]
    bv = bqkv[2 * D : 3 * D]
    wq = w_qkv[:, 0:D] / np.sqrt(DH).astype(np.float32)
    wk = w_qkv[:, D : 2 * D]
    wv = w_qkv[:, 2 * D : 3 * D]
    wv_aug = np.zeros((D, cfg.VA), np.float32)
    vb_aug = np.zeros((cfg.VA,), np.float32)
    for h in range(H):
        wv_aug[:, h * (DH + 1) : h * (DH + 1) + DH] = wv[:, h * DH : (h + 1) * DH]
        vb_aug[h * (DH + 1) : h * (DH + 1) + DH] = bv[h * DH : (h + 1) * DH]
        vb_aug[h * (DH + 1) + DH] = 1.0 / 4096.0
    bfc1 = np.asarray(ln2_b, np.float32) @ np.asarray(w_fc1, np.float32)
    wfc1 = np.asarray(w_fc1, np.float32) * np.asarray(ln2_g, np.float32)[:, None]

    wd = np.float32 if cfg.mmdt == "float32" else np.float16
    weights = {
        "wq": wq.astype(wd),
        "wk": wk.astype(wd),
        "wv": wv_aug.astype(wd),
        "bq": bq.astype(np.float32),
        "bk": bk.astype(np.float32),
        "vb": vb_aug.astype(np.float32),
        "wproj": np.asarray(w_proj, np.float32).astype(wd),
        "wfc1": wfc1.astype(wd),
        "bfc1": bfc1.astype(np.float32),
        "wfc2": np.asarray(w_fc2, np.float32).astype(wd),
    }

    in_maps = []
    core_rows = []
    n_j = 4  # chunk pairs per batch
    for c in range(8):
        b, j = c // n_j, c % n_j
        qgA, qgB = cfg.CH * j, cfg.CH * (2 * n_j - 1 - j)
        rows = np.r_[qgA : qgA + cfg.CH, qgB : qgB + cfg.CH]
        core_rows.append((b, rows))
        im = dict(weights)
        im["x_q"] = np.ascontiguousarray(x[b][rows])
        im["masks"] = make_masks(cfg, qgA, qgB).astype(wd)
        in_maps.append(im)

    def assemble(results):
        out = np.zeros((B, x.shape[1], D), np.float32)
        for c, (b, rows) in enumerate(core_rows):
            out[b][rows] = results[c]["out"]
        return out

    return in_maps, assemble


# ======================= public entry point =======================

LAST_RESULTS = {}
_CACHE = {}


def kernel(x, ln1_g, ln1_b, w_qkv, w_proj, ln2_g, ln2_b, w_fc1, w_fc2,
           _trace=False):
    """Full-input decoder block on 8 TRN2 NeuronCores; returns full output."""
    from concourse.bass_utils import run_bass_kernel_spmd

    cfg = Cfg()
    in_maps, assemble = host_prep(
        cfg, x, ln1_g, ln1_b, w_qkv, w_proj, ln2_g, ln2_b, w_fc1, w_fc2
    )
    if "nc" not in _CACHE:
        _CACHE["nc"] = build(cfg)
    res = run_bass_kernel_spmd(
        _CACHE["nc"], in_maps, core_ids=list(range(8)), trace=_trace
    )
    LAST_RESULTS["res"] = res
    return assemble(res.results)


# revision 17
# speedup vs baseline: 1.0383x; 1.0383x over previous
"""Decoder block Bass/Tile kernel for TRN2, SPMD over 8 cores.

Sharding: core c = (batch b = c//4, j = c%4). Each core:
  - owns 512 query rows of its batch: chunk A = [256j, 256j+256),
    chunk B = [256(7-j), 256(7-j)+256)  (causal zigzag load balance)
  - computes LN1 + K,V for exactly its own 512 rows, then AllGathers
    K,V across the 4 cores of its batch (replica groups [0-3], [4-7])
  - attention klen padded to a uniform size (1024 for A, 2048 for B) with
    host-provided -60000 masks so the program is identical on all cores
  - proj + residual + LN2 + MLP + residual for its 512 rows
Host gathers the 8 [512, 1024] shards into the full output.

Layouts: "fm" = [feature(partition), token(free)], "rm" = [token, feature].
LN in rm (bn_stats), matmul inputs fm via fp16 DMA-transpose. Matmuls fp16
with fp32 PSUM accumulation. Residual stream fp32.
"""

from contextlib import ExitStack
from dataclasses import dataclass

import numpy as np

import concourse.bass as bass
import concourse.tile as tile
from concourse import mybir
from concourse._compat import with_exitstack

F32 = mybir.dt.float32
F16 = mybir.dt.float16
MASK_NEG = -60000.0


@dataclass
class Cfg:
    D: int = 1024
    DFF: int = 4096
    H: int = 16  # heads
    DH: int = 64  # head dim
    T_kv: int = 2048
    T_q: int = 512  # 2 chunks of CH
    CH: int = 256
    klenA_pad: int = 1024
    klenB_pad: int = 2048
    mmdt: str = "float16"

    @property
    def HP(self):  # head pairs
        return self.H // 2

    @property
    def VA(self):  # augmented V width (dv + ones column per head)
        return self.H * (self.DH + 1)

    @property
    def NKTA(self):
        return self.klenA_pad // 128

    @property
    def NKTB(self):
        return self.klenB_pad // 128

    @property
    def NKT(self):
        return self.NKTA + self.NKTB


def _bcast_ap(ap, p=128):
    """[N] dram AP -> [p, N] with partition stride 0."""
    return bass.AP(tensor=ap.tensor, offset=ap.offset, ap=[[0, p]] + list(ap.ap))


@with_exitstack
def decoder_kernel(ctx: ExitStack, tc: tile.TileContext, cfg: Cfg, io: dict):
    nc = tc.nc
    MD = getattr(mybir.dt, cfg.mmdt)
    D, DFF, H, DH = cfg.D, cfg.DFF, cfg.H, cfg.DH
    HP, VA, CH = cfg.HP, cfg.VA, cfg.CH
    T_kv, T_q = cfg.T_kv, cfg.T_q
    ND = D // 128  # feature tiles
    NFF = DFF // 128
    NTKV = T_kv // 128
    NTQ = T_q // 128
    W2 = 2 * CH  # paired-head free width (512)
    RANKS = 4  # cores per batch group

    # V chunk width for psum (<=512); VA = H*65
    n_vch = (VA + 511) // 512
    while VA % n_vch != 0:
        n_vch += 1
    VCH = VA // n_vch
    assert VCH <= 512

    const = ctx.enter_context(tc.tile_pool(name="const", bufs=1))
    eps_t = const.tile([128, 1], F32)
    nc.vector.memset(eps_t, 1e-5)
    bq_sb = const.tile([128, ND], F32)
    nc.gpsimd.dma_start(out=bq_sb, in_=io["bq"].rearrange("(t p) -> p t", p=128))
    bk_sb = const.tile([128, ND], F32)
    nc.gpsimd.dma_start(out=bk_sb, in_=io["bk"].rearrange("(t p) -> p t", p=128))
    bfc1_sb = const.tile([128, NFF], F32)
    nc.gpsimd.dma_start(out=bfc1_sb, in_=io["bfc1"].rearrange("(t p) -> p t", p=128))
    vb_sb = const.tile([128, VA], F32)
    nc.gpsimd.dma_start(out=vb_sb, in_=_bcast_ap(io["vb"]))

    # ---------------- persistent activations ----------------
    acts = ctx.enter_context(tc.tile_pool(name="acts", bufs=1))
    K_sb = [acts.tile([128, T_kv], MD, tag=f"K{d}", name=f"K{d}") for d in range(ND)]
    Q_sb = [acts.tile([128, 2 * T_q], MD, tag=f"Q{d}", name=f"Q{d}") for d in range(ND)]
    for d in range(ND):
        nc.vector.memset(Q_sb[d], 0.0)
    V_all = acts.tile([128, NTKV, VA], MD, tag="Vall", name="Vall")
    O_sb = [acts.tile([128, T_q], MD, tag=f"O{h}", name=f"O{h}") for h in range(HP)]

    # AllGather bounce buffers (DRAM): two phases, A-half then B-half of
    # the local tokens (K [1024,256] + V [256,1040] each, packed flat).
    # Chunk-A attention starts after AG1 while AG2 is still in flight.
    KELEM = D * 256  # 262144
    VELEM = 256 * VA  # 266240
    KVELEM = KELEM + VELEM  # 528384
    assert KVELEM % 512 == 0
    kvA_in = nc.dram_tensor("kvA_in", [KVELEM // 512, 512], MD).ap()
    kvA_out = nc.dram_tensor("kvA_out", [RANKS * KVELEM // 512, 512], MD).ap()
    kvB_in = nc.dram_tensor("kvB_in", [KVELEM // 512, 512], MD).ap()
    kvB_out = nc.dram_tensor("kvB_out", [RANKS * KVELEM // 512, 512], MD).ap()
    GROUPS = [[0, 1, 2, 3], [4, 5, 6, 7]]

    # ---------------- LN + transpose helper ----------------
    def ln_transpose(src_dram, src_sb, n_tiles, fm_tiles, pool, stats, tagp):
        for rt in range(n_tiles):
            if src_dram is not None:
                x_t = pool.tile([128, D], F32, tag=f"{tagp}_in")
                nc.gpsimd.dma_start(
                    out=x_t, in_=src_dram[rt * 128 : (rt + 1) * 128, :]
                )
            else:
                x_t = src_sb[rt]
            nsub = D // 512
            st = stats.tile([128, nsub, 6], F32, tag="ln_st")
            for s in range(nsub):
                nc.vector.bn_stats(
                    out=st[:, s, :], in_=x_t[:, s * 512 : (s + 1) * 512]
                )
            mv = stats.tile([128, 2], F32, tag="ln_mv")
            nc.vector.bn_aggr(out=mv, in_=st)
            sd = stats.tile([128, 1], F32, tag="ln_sd")
            nc.scalar.activation(
                out=sd, in_=mv[:, 1:2],
                func=mybir.ActivationFunctionType.Sqrt, bias=eps_t,
            )
            rec = stats.tile([128, 1], F32, tag="ln_rec")
            nc.vector.reciprocal(out=rec, in_=sd)
            xh = pool.tile([128, D], MD, tag=f"{tagp}_xh")
            nc.vector.tensor_scalar(
                out=xh, in0=x_t, scalar1=mv[:, 0:1], scalar2=rec,
                op0=mybir.AluOpType.subtract, op1=mybir.AluOpType.mult,
            )
            for d in range(ND):
                dst = fm_tiles[d][rt // 4][:, (rt % 4) * 128 : (rt % 4 + 1) * 128]
                eng = nc.sync if (rt * ND + d) % 2 == 0 else nc.scalar
                eng.dma_start_transpose(
                    out=dst, in_=xh[:, d * 128 : (d + 1) * 128]
                )

    mpA = ctx.enter_context(tc.tile_pool(name="attn_mA", bufs=1))
    with tc.tile_pool(name="fm", bufs=1) as fmp:
        xq_fm = [
            [fmp.tile([128, 512], MD, tag=f"xqfm{d}_0", name=f"xqfm{d}_0")]
            for d in range(ND)
        ]
        with tc.tile_pool(name="wqk", bufs=1) as wqk, tc.tile_pool(
            name="wv", bufs=1
        ) as wvp, tc.tile_pool(name="kvloc", bufs=1) as kvp, tc.tile_pool(
            name="psqkv", bufs=4, space="PSUM"
        ) as psq:
            # LN1 first: its x_q loads must be at the HEAD of the gpsimd
            # DMA queue (weights queue behind them) so K/V/AG start early
            with tc.tile_pool(name="ln1", bufs=3) as lnp, tc.tile_pool(
                name="ln1st", bufs=4
            ) as lnst:
                ln_transpose(io["x_q"], None, NTQ, xq_fm, lnp, lnst, "q")

            # chunk-A attention masks: preload on scalar so they don't
            # queue behind the AG-gated readbacks on sync
            masksA = [
                mpA.tile([128, W2], MD, tag=f"mask0_{k}", name=f"mask0_{k}")
                for k in range(cfg.NKTA)
            ]
            for k in range(cfg.NKTA):
                nc.scalar.dma_start(out=masksA[k], in_=io["masks"][k, :, :])

            wv_sb = [wvp.tile([128, VA], MD, tag=f"wv{kt}", name=f"wv{kt}") for kt in range(ND)]
            for kt in range(ND):
                nc.gpsimd.dma_start(
                    out=wv_sb[kt], in_=io["wv"][kt * 128 : (kt + 1) * 128, :]
                )
            wk_sb = [wqk.tile([128, ND, 128], MD, tag=f"wk{do}", name=f"wk{do}") for do in range(ND)]
            wq_sb = [wqk.tile([128, ND, 128], MD, tag=f"wq{do}", name=f"wq{do}") for do in range(ND)]
            for do in range(ND):
                nc.gpsimd.dma_start(
                    out=wk_sb[do],
                    in_=io["wk"][:, do * 128 : (do + 1) * 128].rearrange(
                        "(kt p) c -> p kt c", p=128
                    ),
                )
            for do in range(ND):
                nc.gpsimd.dma_start(
                    out=wq_sb[do],
                    in_=io["wq"][:, do * 128 : (do + 1) * 128].rearrange(
                        "(kt p) c -> p kt c", p=128
                    ),
                )
            kloc = kvp.tile([128, ND, 512], MD, tag="kloc", name="kloc")
            vloc = kvp.tile([128, NTQ, VA], MD, tag="vloc", name="vloc")

            # V local (own 512 tokens, rm layout)
            for tt in range(NTQ):
                for ch in range(n_vch):
                    ps = psq.tile([128, VCH], F32, tag="psv")
                    for kt in range(ND):
                        nc.tensor.matmul(
                            ps,
                            xq_fm[kt][0][:, tt * 128 : (tt + 1) * 128],
                            wv_sb[kt][:, ch * VCH : (ch + 1) * VCH],
                            start=(kt == 0),
                            stop=(kt == ND - 1),
                        )
                    nc.vector.tensor_add(
                        out=vloc[:, tt, ch * VCH : (ch + 1) * VCH],
                        in0=ps,
                        in1=vb_sb[:, ch * VCH : (ch + 1) * VCH],
                    )
            # K local (own 512 tokens, fm layout)
            for do in range(ND):
                ps = psq.tile([128, 512], F32, tag="psqk")
                for kt in range(ND):
                    nc.tensor.matmul(
                        ps, wk_sb[do][:, kt, :], xq_fm[kt][0],
                        start=(kt == 0), stop=(kt == ND - 1),
                    )
                nc.scalar.activation(
                    out=kloc[:, do, :], in_=ps,
                    func=mybir.ActivationFunctionType.Identity,
                    bias=bk_sb[:, do : do + 1],
                )

            # ship local K,V to DRAM and AllGather within the batch group
            # (two phases: A tokens then B tokens)
            for kv_in, c0 in ((kvA_in, 0), (kvB_in, 256)):
                nc.gpsimd.dma_start(
                    out=bass.AP(
                        tensor=kv_in.tensor, offset=kv_in.offset,
                        ap=[[256, 128], [128 * 256, ND], [1, 256]],
                    ),
                    in_=kloc[:, :, c0 : c0 + 256],
                )
                nc.gpsimd.dma_start(
                    out=bass.AP(
                        tensor=kv_in.tensor, offset=kv_in.offset + KELEM,
                        ap=[[VA, 128], [128 * VA, 2], [1, VA]],
                    ),
                    in_=vloc[:, 2 * (c0 // 256) : 2 * (c0 // 256) + 2, :],
                )
            nc.gpsimd.collective_compute(
                "AllGather", mybir.AluOpType.bypass, replica_groups=GROUPS,
                ins=[kvA_in], outs=[kvA_out],
            )
            nc.gpsimd.collective_compute(
                "AllGather", mybir.AluOpType.bypass, replica_groups=GROUPS,
                ins=[kvB_in], outs=[kvB_out],
            )

            # Q (overlaps the AllGather; no gpsimd involvement)
            for do in range(ND):
                ps = psq.tile([128, 512], F32, tag="psqk")
                for kt in range(ND):
                    nc.tensor.matmul(
                        ps, wq_sb[do][:, kt, :], xq_fm[kt][0],
                        start=(kt == 0), stop=(kt == ND - 1),
                    )
                # Q: scatter into per-(chunk, head) blocks with the
                # complementary head's partitions left zero
                for ci in range(2):
                    for h in range(2):
                        blk = (2 * ci + h) * CH
                        nc.scalar.activation(
                            out=Q_sb[do][
                                h * 64 : (h + 1) * 64,
                                blk : blk + CH,
                            ],
                            in_=ps[
                                h * 64 : (h + 1) * 64,
                                ci * CH : (ci + 1) * CH,
                            ],
                            func=mybir.ActivationFunctionType.Identity,
                            bias=bq_sb[h * 64 : (h + 1) * 64, do : do + 1],
                        )

            # readback on the sync queue (gpsimd is blocked on the AG2
            # wait).  Phase A: K_sb cols 0..1024 + V tiles 0..7 (everything
            # chunk-A attention needs).  Phase B: the rest (waits AG2).
            # rank r local tokens = [A: 256r..256r+256) | B: 256(7-r)..+256)
            for do in range(ND):
                src = bass.AP(
                    tensor=kvA_out.tensor,
                    offset=kvA_out.offset + do * 128 * 256,
                    ap=[[256, 128], [KVELEM, RANKS], [1, 256]],
                )
                nc.sync.dma_start(
                    out=K_sb[do][:, 0:1024].rearrange("p (r c) -> p r c", r=4),
                    in_=src,
                )
            for r in range(RANKS):
                src = bass.AP(
                    tensor=kvA_out.tensor,
                    offset=kvA_out.offset + r * KVELEM + KELEM,
                    ap=[[VA, 128], [128 * VA, 2], [1, VA]],
                )
                nc.sync.dma_start(out=V_all[:, 2 * r : 2 * r + 2, :], in_=src)
            for do in range(ND):
                for r in range(RANKS):
                    src = bass.AP(
                        tensor=kvB_out.tensor,
                        offset=kvB_out.offset + r * KVELEM + do * 128 * 256,
                        ap=[[256, 128], [1, 256]],
                    )
                    nc.sync.dma_start(
                        out=K_sb[do][:, 256 * (7 - r) : 256 * (8 - r)], in_=src
                    )
            for r in range(RANKS):
                src = bass.AP(
                    tensor=kvB_out.tensor,
                    offset=kvB_out.offset + r * KVELEM + KELEM,
                    ap=[[VA, 128], [128 * VA, 2], [1, VA]],
                )
                nc.sync.dma_start(
                    out=V_all[:, 14 - 2 * r : 16 - 2 * r, :], in_=src
                )

    # ---------------- attention + proj ----------------
    mid = ctx.enter_context(tc.tile_pool(name="mid", bufs=1))
    x2_sb = [mid.tile([128, D], F32, tag=f"x2_{t}", name=f"x2_{t}") for t in range(NTQ)]
    xq2_fm = [
        [mid.tile([128, 512], MD, tag=f"xq2fm{d}_{c}", name=f"xq2fm{d}_{c}")
         for c in range(T_q // 512)]
        for d in range(ND)
    ]
    rscr = nc.dram_tensor("rscratch", [2 * HP * 2, CH], F32).ap()
    chunks = [(0, cfg.NKTA, 0), (1, cfg.NKTB, cfg.NKTA)]  # (ci, nkt, mask_off)
    with tc.tile_pool(name="attn_w", bufs=1) as awp:
        # prefetch wproj on gpsimd (free once AG2 completes, well before proj)
        wproj_sb = [awp.tile([128, D], MD, tag=f"wp{d}", name=f"wp{d}") for d in range(ND)]
        for d in range(ND):
            nc.gpsimd.dma_start(
                out=wproj_sb[d], in_=io["wproj"][d * 128 : (d + 1) * 128, :]
            )
        with tc.tile_pool(name="attn_m", bufs=1) as mp, tc.tile_pool(
            name="attn_p", bufs=4
        ) as pp, tc.tile_pool(name="attn_ps", bufs=4, space="PSUM"
        ) as aps, tc.tile_pool(name="attn_po", bufs=4, space="PSUM"
        ) as ops:
            for ci, nkt, moff in chunks:
                cc = slice(ci * CH, (ci + 1) * CH)
                if ci == 0:
                    masks = masksA
                else:
                    masks = []
                    for k in range(nkt):
                        m = mp.tile([128, W2], MD, tag=f"mask{ci}_{k}")
                        nc.sync.dma_start(out=m, in_=io["masks"][moff + k, :, :])
                        masks.append(m)
                for hp in range(HP):
                    po = [ops.tile([128, CH], F32, tag="po", name="po") for _ in range(2)]
                    for kti in range(nkt):
                        ps = aps.tile([128, W2], F32, tag="ps_s")
                        kcol = slice(kti * 128, (kti + 1) * 128)
                        nc.tensor.matmul(
                            ps,
                            K_sb[hp][:, kcol],
                            Q_sb[hp][:, 2 * ci * CH : 2 * ci * CH + W2],
                            start=True, stop=True,
                        )
                        if not (ci == 1 and (kti + 1) * 128 <= cfg.klenB_pad // 2):
                            nc.vector.tensor_add(
                                out=ps, in0=ps, in1=masks[kti]
                            )
                        pt = pp.tile([128, W2], MD, tag="pt")
                        nc.scalar.activation(
                            out=pt, in_=ps,
                            func=mybir.ActivationFunctionType.Exp,
                        )
                        for h in range(2):
                            hg = 2 * hp + h
                            nc.tensor.matmul(
                                po[h][0:65, :],
                                V_all[:, kti, hg * 65 : hg * 65 + 65],
                                pt[:, h * CH : (h + 1) * CH],
                                start=(kti == 0),
                                stop=(kti == nkt - 1),
                            )
                    # normalize + evict (inline: the DRAM broadcast
                    # round-trip hides behind the next hp's attention)
                    for h in range(2):
                        slot = (ci * HP + hp) * 2 + h
                        r = pp.tile([1, CH], F32, tag="recip")
                        nc.vector.reciprocal(out=r, in_=po[h][64:65, :])
                        nc.sync.dma_start(
                            out=rscr[slot : slot + 1, :], in_=r
                        )
                        # evict numerator scaled by 1/4096 (fits fp16)
                        nc.scalar.activation(
                            out=O_sb[hp][h * 64 : (h + 1) * 64, cc],
                            in_=po[h][0:64, :],
                            func=mybir.ActivationFunctionType.Copy,
                            scale=1.0 / 4096.0,
                        )
                    bc_sb = pp.tile([128, CH], F32, tag="bcsb")
                    for h in range(2):
                        slot = (ci * HP + hp) * 2 + h
                        nc.sync.dma_start(
                            out=bc_sb[h * 64 : (h + 1) * 64, :],
                            in_=bass.AP(
                                tensor=rscr.tensor,
                                offset=rscr.offset + slot * CH,
                                ap=[[0, 64], [1, CH]],
                            ),
                        )
                    nc.vector.tensor_mul(
                        out=O_sb[hp][:, cc], in0=O_sb[hp][:, cc], in1=bc_sb
                    )

        # ---------------- proj + residual ----------------
        with tc.tile_pool(name="proj", bufs=3) as prp, tc.tile_pool(
            name="projps", bufs=4, space="PSUM"
        ) as prps:
            for qt in range(NTQ):
                x_t = prp.tile([128, D], F32, tag="xq_res")
                nc.gpsimd.dma_start(
                    out=x_t, in_=io["x_q"][qt * 128 : (qt + 1) * 128, :]
                )
                for ch2 in range(D // 512):
                    ps = prps.tile([128, 512], F32, tag="pspr")
                    for hp in range(ND):
                        nc.tensor.matmul(
                            ps,
                            O_sb[hp][:, qt * 128 : (qt + 1) * 128],
                            wproj_sb[hp][:, ch2 * 512 : (ch2 + 1) * 512],
                            start=(hp == 0),
                            stop=(hp == ND - 1),
                        )
                    nc.vector.tensor_add(
                        out=x2_sb[qt][:, ch2 * 512 : (ch2 + 1) * 512],
                        in0=ps,
                        in1=x_t[:, ch2 * 512 : (ch2 + 1) * 512],
                    )

    # ---------------- LN2 + transpose ----------------
    with tc.tile_pool(name="ln2", bufs=3) as ln2p, tc.tile_pool(
        name="ln2st", bufs=4
    ) as ln2st:
        ln_transpose(None, x2_sb, NTQ, xq2_fm, ln2p, ln2st, "l2")

    # ---------------- fc1 + gelu + fc2 (pipelined) ----------------
    ghp = ctx.enter_context(tc.tile_pool(name="gh", bufs=1))
    gh_sb = [ghp.tile([128, T_q], MD, tag=f"gh{f}", name=f"gh{f}") for f in range(NFF)]
    with tc.tile_pool(name="fc1w", bufs=3) as f1w, tc.tile_pool(
        name="fc2w", bufs=3
    ) as f2w, tc.tile_pool(name="fc2out", bufs=3) as f2o, tc.tile_pool(
        name="fcps", bufs=3, space="PSUM"
    ) as fps, tc.tile_pool(name="fc2acc", bufs=1, space="PSUM") as f2ps:
        for sweep in range(2):
            accs = {}
            for qt in range(NTQ):
                accs[qt] = f2ps.tile(
                    [128, 512], F32, tag=f"acc{qt}", name=f"acc{qt}"
                )
            for ff in range(NFF):
                if sweep == 0:
                    wb = f1w.tile([128, ND, 128], MD, tag="wfc1")
                    nc.gpsimd.dma_start(
                        out=wb,
                        in_=io["wfc1"][:, ff * 128 : (ff + 1) * 128].rearrange(
                            "(kt p) c -> p kt c", p=128
                        ),
                    )
                    ps = fps.tile([128, T_q], F32, tag="psf1")
                    for kt in range(ND):
                        nc.tensor.matmul(
                            ps, wb[:, kt, :], xq2_fm[kt][0],
                            start=(kt == 0), stop=(kt == ND - 1),
                        )
                    nc.scalar.activation(
                        out=gh_sb[ff], in_=ps,
                        func=mybir.ActivationFunctionType.Gelu,
                        bias=bfc1_sb[:, ff : ff + 1],
                    )
                wb2 = f2w.tile([128, 512], MD, tag="wfc2")
                nc.gpsimd.dma_start(
                    out=wb2,
                    in_=io["wfc2"][
                        ff * 128 : (ff + 1) * 128, sweep * 512 : (sweep + 1) * 512
                    ],
                )
                for qt in range(NTQ):
                    nc.tensor.matmul(
                        accs[qt],
                        gh_sb[ff][:, qt * 128 : (qt + 1) * 128],
                        wb2,
                        start=(ff == 0),
                        stop=(ff == NFF - 1),
                    )
            for qt in range(NTQ):
                o = f2o.tile([128, 512], F32, tag="osb")
                nc.vector.tensor_add(
                    out=o,
                    in0=accs[qt],
                    in1=x2_sb[qt][:, sweep * 512 : (sweep + 1) * 512],
                )
                nc.sync.dma_start(
                    out=io["out"][
                        qt * 128 : (qt + 1) * 128,
                        sweep * 512 : (sweep + 1) * 512,
                    ],
                    in_=o,
                )


def split_drain_waits(nc):
    """walrus CoreV3 rejects >1 sync wait on several instruction types;
    split extras into single-wait NOPs preceding the instruction on the
    same (in-order) engine."""
    idx = 0

    def fix_block(b):
        nonlocal idx
        new = []
        changed = False
        for inst in b.instructions:
            si = inst.sync_info
            if si is not None and si.on_wait and len(si.on_wait) > 1:
                waits = list(si.on_wait)
                for w in waits[:-1]:
                    idx += 1
                    nop = mybir.InstNoOp(
                        name=f"I-dsplit-{idx}",
                        sync_info=mybir.SyncInfo(on_wait=[w], on_update=[]),
                    )
                    nop.engine = inst.engine
                    new.append(nop)
                inst.sync_info = mybir.SyncInfo(
                    on_wait=[waits[-1]], on_update=list(si.on_update or [])
                )
                changed = True
            new.append(inst)
        if changed:
            b.instructions = new

    for f in nc.m.functions:
        for b in f.blocks:
            fix_block(b)


def declare_io(nc, cfg: Cfg):
    c = cfg
    WD = getattr(mybir.dt, c.mmdt)
    spec = {
        "x_q": ([c.T_q, c.D], F32, False),
        "wq": ([c.D, c.D], WD, False),
        "wk": ([c.D, c.D], WD, False),
        "wv": ([c.D, c.VA], WD, False),
        "bq": ([c.D], F32, False),
        "bk": ([c.D], F32, False),
        "vb": ([c.VA], F32, False),
        "wproj": ([c.D, c.D], WD, False),
        "wfc1": ([c.D, c.DFF], WD, False),
        "bfc1": ([c.DFF], F32, False),
        "wfc2": ([c.DFF, c.D], WD, False),
        "masks": ([c.NKT, 128, 2 * c.CH], WD, False),
        "out": ([c.T_q, c.D], F32, True),
    }
    io = {}
    for name, (shape, dt, is_out) in spec.items():
        io[name] = nc.declare_dram_parameter(name, shape, dt, isOutput=is_out).ap()
    return io


def build(cfg: Cfg, split: bool = True):
    nc = bass.Bass(num_devices=8)
    io = declare_io(nc, cfg)
    with tile.TileContext(nc) as tc:
        decoder_kernel(tc, cfg, io)
    if split:
        split_drain_waits(nc)
    return nc


# ======================= host-side prep =======================


def make_masks(cfg: Cfg, qgA, qgB):
    """[NKT, 128, 2*CH] fp16-ish: 0 where key k <= query q (valid), else
    -60000. Duplicated for the 2 heads along the free dim."""
    m = np.full((cfg.NKT, 128, 2 * cfg.CH), MASK_NEG, np.float32)
    for ci, (qg, nkt, off) in enumerate(
        [(qgA, cfg.NKTA, 0), (qgB, cfg.NKTB, cfg.NKTA)]
    ):
        q = qg + np.arange(cfg.CH)[None, :]
        for k in range(nkt):
            kg = k * 128 + np.arange(128)[:, None]
            valid = (kg <= q).astype(np.float32)
            blk = (1.0 - valid) * MASK_NEG
            m[off + k, :, 0 : cfg.CH] = blk
            m[off + k, :, cfg.CH : 2 * cfg.CH] = blk
    return m.astype(np.float16)


def host_prep(cfg: Cfg, x, ln1_g, ln1_b, w_qkv, w_proj, ln2_g, ln2_b, w_fc1, w_fc2):
    """Returns (in_maps list of 8 dicts, assemble(results)->full out)."""
    D, H, DH = cfg.D, cfg.H, cfg.DH
    x = np.asarray(x, np.float32)
    B = x.shape[0]
    w_qkv = np.asarray(w_qkv, np.float32)
    bqkv = np.asarray(ln1_b, np.float32) @ w_qkv  # [3D]
    w_qkv = w_qkv * np.asarray(ln1_g, np.float32)[:, None]
    bq = bqkv[0:D] / np.sqrt(DH).astype(np.float32)
    bk = bqkv[D : 2 * D]
    bv = bqkv[2 * D : 3 * D]
    wq = w_qkv[:, 0:D] / np.sqrt(DH).astype(np.float32)
    wk = w_qkv[:, D : 2 * D]
    wv = w_qkv[:, 2 * D : 3 * D]
    wv_aug = np.zeros((D, cfg.VA), np.float32)
    vb_aug = np.zeros((cfg.VA,), np.float32)
    for h in range(H):
        wv_aug[:, h * (DH + 1) : h * (DH + 1) + DH] = wv[:, h * DH : (h + 1) * DH]
        vb_aug[h * (DH + 1) : h * (DH + 1) + DH] = bv[h * DH : (h + 1) * DH]
        vb_aug[h * (DH + 1) + DH] = 1.0 / 4096.0
    bfc1 = np.asarray(ln2_b, np.float32) @ np.asarray(w_fc1, np.float32)
    wfc1 = np.asarray(w_fc1, np.float32) * np.asarray(ln2_g, np.float32)[:, None]

    wd = np.float32 if cfg.mmdt == "float32" else np.float16
    weights = {
        "wq": wq.astype(wd),
        "wk": wk.astype(wd),
        "wv": wv_aug.astype(wd),
        "bq": bq.astype(np.float32),
        "bk": bk.astype(np.float32),
        "vb": vb_aug.astype(np.float32),
        "wproj": np.asarray(w_proj, np.float32).astype(wd),
        "wfc1": wfc1.astype(wd),
        "bfc1": bfc1.astype(np.float32),
        "wfc2": np.asarray(w_fc2, np.float32).astype(wd),
    }

    in_maps = []
    core_rows = []
    n_j = 4  # chunk pairs per batch
    for c in range(8):
        b, j = c // n_j, c % n_j
        qgA, qgB = cfg.CH * j, cfg.CH * (2 * n_j - 1 - j)
        rows = np.r_[qgA : qgA + cfg.CH, qgB : qgB + cfg.CH]
        core_rows.append((b, rows))
        im = dict(weights)
        im["x_q"] = np.ascontiguousarray(x[b][rows])
        im["masks"] = make_masks(cfg, qgA, qgB).astype(wd)
        in_maps.append(im)

    def assemble(results):
        out = np.zeros((B, x.shape[1], D), np.float32)
        for c, (b, rows) in enumerate(core_rows):
            out[b][rows] = results[c]["out"]
        return out

    return in_maps, assemble


# ======================= public entry point =======================

LAST_RESULTS = {}
_CACHE = {}


def kernel(x, ln1_g, ln1_b, w_qkv, w_proj, ln2_g, ln2_b, w_fc1, w_fc2,
           _trace=False):
    """Full-input decoder block on 8 TRN2 NeuronCores; returns full output."""
    from concourse.bass_utils import run_bass_kernel_spmd

    cfg = Cfg()
    in_maps, assemble = host_prep(
        cfg, x, ln1_g, ln1_b, w_qkv, w_proj, ln2_g, ln2_b, w_fc1, w_fc2
    )
    if "nc" not in _CACHE:
        _CACHE["nc"] = build(cfg)
    res = run_bass_kernel_spmd(
        _CACHE["nc"], in_maps, core_ids=list(range(8)), trace=_trace
    )
    LAST_RESULTS["res"] = res
    return assemble(res.results)
